# revision 1
# baseline (speedup 1.0000x reference)
"""MTLU (histogram-binning piecewise-linear unit) Trainium2 kernel.

Math: the reference computes, per channel c and element x,
    idx = clip(floor(x/0.1) + 10, 0, 19)
    out = w[c, idx] * x + b[c, idx]
with w = (y - y_)/0.1, b = y - (y - y_)*index (index = -9..10).

Because y_[:, k] == y[:, k-1] (frozen shifted buffer) the function is a
CONTINUOUS piecewise-linear function of x with uniform breakpoints
t_k = (k-10)/10, k=1..19.  Any such function equals a ReLU sum:
    out = w0[c]*x + b0[c] + sum_{k=1..19} d_k[c] * relu(x - t_k),
    d_k = w[c,k] - w[c,k-1].
No gather / floor / clamp needed.  In the X10 = 10*x domain the
breakpoints become consecutive integers, which lets one custom DVE
instruction evaluate TWO relu terms plus the running accumulator
(8 ALU stages exactly):
    PAIRT: out = Src1 + C0*relu(Src0 - C2) + C1*relu(Src0 - (C2+1))
    BASE3: out = C0*Src0 + C1 + C3*relu(Src0 - C2)   (C3 via Src1 latch)
So one full pass = 1 ACT (X10 = 10*x) + 10 DVE instructions.

Sharding: pure data parallel over batch — 16 batches -> 2 per core x 8
cores.  Per-core layout [2*64, 65536] puts channel on the partition dim
(coefficients become per-partition scalars, replicated x2).
"""

import numpy as np

# problem constants (hardcoded per contract)
B, FEAT, H, W = 16, 64, 256, 256
BIN_NUM, HALF = 20, 10
N_CORES = 8
BPC = B // N_CORES                # batches per core
P = BPC * FEAT                    # 128 partitions
FREE = H * W                      # 65536 free elems per partition
CHUNK = 4096
NCHUNK = FREE // CHUNK
NTERM = BIN_NUM - 1               # 19 relu terms

_STATE: dict = {}


def _register_ops():
    """Register the two custom DVE ops (idempotent)."""
    import concourse.dve_ops as dve_ops
    from concourse.dve_ops import DveOp
    from concourse.dve_spec import (
        C0, C1, C2, C3, One, Spec, Src0, Src1, lower, relu,
        _has_src1, _spill_c3_to_src1,
    )
    from concourse.dve_uop import DveOpSpec

    if "PAIRT_MTLU" in dve_ops._SUB_OPCODE_FOR_NAME:
        by_name = {op.name: op for op in dve_ops.OPS}
        return by_name["PAIRT_MTLU"], by_name["BASE3_MTLU"]

    def _mk(name, spec):
        row = dve_ops._CUSTOM_DVE_ROW_BASE + len(dve_ops.OPS)
        assert row < 0x20
        shas = {}
        for ver in ("v3", "v4"):
            try:
                u = lower(spec, ver=ver)
                shas[ver] = DveOpSpec(
                    name=name, opcode=row, uops=u, rd1_en=_has_src1(spec)
                ).sha(ver)
            except Exception:
                pass
        op = DveOp(name, spec, subdim=False, uops_sha=shas)
        dve_ops.OPS.append(op)
        dve_ops._SUB_OPCODE_FOR_NAME[name] = row
        dve_ops.CUSTOM_DVE_SPECS[name] = spec
        return op

    def _ref_pair(in0, in1, s0, s1, imm2):
        a = in0 - imm2
        return in1 + s0 * np.maximum(a, 0) + s1 * np.maximum(a - 1.0, 0)

    def _ref_base(in0, in1, s0, s1, imm2):
        return s0 * in0 + s1 + in1 * np.maximum(in0 - imm2, 0)

    pair = _mk(
        "PAIRT_MTLU",
        Spec(
            body=Src1 + C0 * relu(Src0 - C2) + C1 * relu(Src0 - (C2 + One)),
            reference=_ref_pair,
        ),
    )
    base = _mk(
        "BASE3_MTLU",
        Spec(
            body=_spill_c3_to_src1(C0 * Src0 + C1 + C3 * relu(Src0 - C2)),
            reference=_ref_base,
        ),
    )
    return pair, base


def _build_module():
    import concourse.bacc as bacc
    import concourse.tile as tile
    from concourse import mybir

    PAIRT, BASE3 = _register_ops()

    nc = bacc.Bacc(
        "TRN2", target_bir_lowering=False, debug=False, num_devices=N_CORES
    )
    f32 = mybir.dt.float32
    x_in = nc.dram_tensor("x", [P, FREE], f32, kind="ExternalInput")
    coef = nc.dram_tensor("coef", [P, 1 + 1 + NTERM], f32, kind="ExternalInput")
    out = nc.dram_tensor("out", [P, FREE], f32, kind="ExternalOutput")

    with tile.TileContext(nc) as tc:
        with (
            tc.tile_pool(name="coefp", bufs=1) as cpool,
            tc.tile_pool(name="xp", bufs=3) as xpool,
            tc.tile_pool(name="x10p", bufs=2) as x10pool,
            tc.tile_pool(name="accp", bufs=4) as accpool,
        ):
            ct = cpool.tile([P, 1 + 1 + NTERM], f32)
            nc.sync.dma_start(ct[:], coef[:])
            for i in range(NCHUNK):
                sl = slice(i * CHUNK, (i + 1) * CHUNK)
                xr = xpool.tile([P, CHUNK], f32, tag="xr")
                nc.sync.dma_start(xr[:], x_in[:, sl])
                x10 = x10pool.tile([P, CHUNK], f32, tag="x10")
                nc.scalar.activation(
                    x10[:], xr[:], mybir.ActivationFunctionType.Copy, scale=10.0
                )
                # base affine + term 1  (t'_1 = -9)
                acc = accpool.tile([P, CHUNK], f32, tag="acc")
                nc.vector._custom_dve(
                    BASE3,
                    out=acc[:],
                    in0=x10[:],
                    in1=ct[:, 2:3],
                    s0=ct[:, 0:1],
                    s1=ct[:, 1:2],
                    imm2=-9.0,
                )
                # 9 pair passes: terms (2+2j, 3+2j), t'_{2+2j} = 2j-8
                for j in range(9):
                    nxt = accpool.tile([P, CHUNK], f32, tag="acc")
                    nc.vector._custom_dve(
                        PAIRT,
                        out=nxt[:],
                        in0=x10[:],
                        in1=acc[:],
                        s0=ct[:, 3 + 2 * j : 4 + 2 * j],
                        s1=ct[:, 4 + 2 * j : 5 + 2 * j],
                        imm2=float(2 * j - 8),
                    )
                    acc = nxt
                nc.sync.dma_start(out[:, sl], acc[:])

    nc.compile()
    return nc


def _coef_table(mtlu_y: np.ndarray, mtlu_y_: np.ndarray) -> np.ndarray:
    y = mtlu_y.astype(np.float32)
    y_ = mtlu_y_.astype(np.float32)
    index = (np.arange(BIN_NUM) - (HALF - 1)).astype(np.float32)  # -9..10
    w = ((y - y_) / np.float32(0.1)).astype(np.float32)
    b = (y - (y - y_) * index).astype(np.float32)
    d = (w[:, 1:] - w[:, :-1]).astype(np.float32)                 # [64,19]
    c = np.concatenate(
        [w[:, :1] / np.float32(10.0), b[:, :1], d / np.float32(10.0)], axis=1
    ).astype(np.float32)                                          # [64,21]
    return np.tile(c, (BPC, 1))                                   # [128,21]


def kernel(x: np.ndarray, mtlu_y: np.ndarray, mtlu_y_: np.ndarray) -> np.ndarray:
    from concourse.bass_utils import run_bass_kernel_spmd

    if "nc" not in _STATE:
        _STATE["nc"] = _build_module()
    nc = _STATE["nc"]

    coef = _coef_table(np.asarray(mtlu_y), np.asarray(mtlu_y_))
    xs = np.ascontiguousarray(x, dtype=np.float32).reshape(B, FEAT, FREE)
    in_maps = [
        {"x": xs[i * BPC : (i + 1) * BPC].reshape(P, FREE), "coef": coef}
        for i in range(N_CORES)
    ]
    res = run_bass_kernel_spmd(
        nc,
        in_maps,
        core_ids=list(range(N_CORES)),
        trace=bool(int(__import__("os").environ.get("MTLU_TRACE", "0"))),
    )
    _STATE["last_results"] = res
    out = np.concatenate(
        [r["out"].reshape(BPC, FEAT, H, W) for r in res.results], axis=0
    )
    return out
